# revision 12
# baseline (speedup 1.0000x reference)
"""PointTransformerLayer (multi-head) Trainium2 Bass kernel, 8-core SPMD.

Sharding: N=30000 points split into 8 contiguous shards of 3750 (whole
point-clouds stay on groups of 4 cores; each core replicates its cloud's
k/v tables so every KNN gather is core-local).

Per-core device pipeline (feature-major layout: partitions = 128 features,
free = (point, neighbor) columns):
  - build bf16 packed [k|v] token table in SBUF via point-major matmuls
    (centering of k/q and all biases are folded into host-prepared weights)
  - dma_gather (SBUF-source, transpose mode) pulls 16 neighbor rows per
    point as feature-major bf16 planes
  - r = kgc + pec - xqc assembled in PSUM by accumulating matmuls
  - LN over d (per head) via block-diagonal 1/16 matmul of squares,
    sqrt on ACT, fast reciprocal on DVE
  - attention-weight MLP as block-diagonal 128x128 matmuls
  - softmax over the 16 neighbors: exp on ACT, strided free-axis reduce,
    normalization deferred to after the weighted neighbor sum
"""

import os
import sys

import numpy as np

for _p in ("/opt/trn_rl_repo", "/root/.axon_site/_ro/trn_rl_repo"):
    if os.path.isdir(_p) and _p not in sys.path:
        sys.path.insert(0, _p)

import ml_dtypes

BF16 = ml_dtypes.bfloat16

# Problem constants (hardcoded per harness contract)
N, C, O, H, S = 30000, 128, 128, 8, 16
D = O // H          # 16
OS = 16
EPS = 1e-5
N_CORES = 8
OFFSETS = (15000, 30000)

DEF_CFG = dict(n=N, n_cores=N_CORES, offsets=OFFSETS)


def _derive(cfg):
    n = cfg["n"]
    n_cores = cfg["n_cores"]
    offsets = cfg["offsets"]
    clouds = len(offsets)
    cloud = offsets[0]
    for i, e in enumerate(offsets):
        assert e == (i + 1) * cloud, "equal-size clouds required"
    assert n == offsets[-1]
    cores_per_cloud = n_cores // clouds
    assert cores_per_cloud * clouds == n_cores
    np_ = n // n_cores                      # points per core
    pts_pad = -(-np_ // 128) * 128          # padded points per core
    nblk = pts_pad // 128
    ns_pad = pts_pad * S
    tpts = -(-cloud // 512) * 512           # padded table tokens
    ranks = tpts // 128
    return dict(
        n=n, n_cores=n_cores, clouds=clouds, cloud=cloud,
        cores_per_cloud=cores_per_cloud, np_=np_, pts_pad=pts_pad,
        nblk=nblk, ns_pad=ns_pad, tpts=tpts, ranks=ranks,
    )


# --------------------------------------------------------------------------
# Device module
# --------------------------------------------------------------------------

def build_module(cfg=None, stage=None):
    import os as _os
    if stage is None:
        stage = int(_os.environ.get("BASS_STAGE", "5"))
    import concourse.bacc as bacc
    import concourse.mybir as mybir
    import concourse.tile as tile
    from concourse.library_config import mlp

    d = _derive(cfg or DEF_CFG)
    TPTS, RANKS, PTS_PAD, NBLK, NS_PAD = (
        d["tpts"], d["ranks"], d["pts_pad"], d["nblk"], d["ns_pad"])

    dt = mybir.dt
    f32, bf16, i16 = dt.float32, dt.bfloat16, dt.int16
    AF = mybir.ActivationFunctionType
    ALU = mybir.AluOpType
    AX = mybir.AxisListType

    nc = bacc.Bacc("TRN2", target_bir_lowering=False, debug=False,
                   enable_asserts=False)

    xT_d = nc.dram_tensor("xT", [128, TPTS], bf16, kind="ExternalInput")
    rz4_d = nc.dram_tensor("rz4", [4, NS_PAD], bf16, kind="ExternalInput")
    gidx_d = nc.dram_tensor("gidx", [128, PTS_PAD], i16, kind="ExternalInput")
    wkc_d = nc.dram_tensor("Wkc", [128, 128], bf16, kind="ExternalInput")
    wqn_d = nc.dram_tensor("Wqn", [128, 128], bf16, kind="ExternalInput")
    wv_d = nc.dram_tensor("Wv", [128, 128], bf16, kind="ExternalInput")
    wp2c_d = nc.dram_tensor("Wp2cx", [4, 128], bf16, kind="ExternalInput")
    wp2_d = nc.dram_tensor("Wp2x", [4, 128], bf16, kind="ExternalInput")
    bd1_d = nc.dram_tensor("BD1", [128, 128], bf16, kind="ExternalInput")
    bd16_d = nc.dram_tensor("BD16", [128, 128], bf16, kind="ExternalInput")
    bd2_d = nc.dram_tensor("BD2", [128, 128], bf16, kind="ExternalInput")
    i128_d = nc.dram_tensor("I128", [128, 128], bf16, kind="ExternalInput")
    bd16f_d = nc.dram_tensor("BD16f", [128, 128], f32, kind="ExternalInput")
    bd2f_d = nc.dram_tensor("BD2f", [128, 128], f32, kind="ExternalInput")
    vecs_d = nc.dram_tensor("vecs", [128, 6], f32, kind="ExternalInput")
    out_d = nc.dram_tensor("outT", [128, PTS_PAD], f32, kind="ExternalOutput")

    with tile.TileContext(nc) as tc:
        with (
            tc.tile_pool(name="const", bufs=1) as const,
            tc.tile_pool(name="big", bufs=1) as big,
        ):
            nc.gpsimd.load_library(mlp)

            def cload(dram, shape, dtp):
                t = const.tile(shape, dtp, tag=dram.name)
                nc.sync.dma_start(t[:], dram[:])
                return t

            Wkc = cload(wkc_d, [128, 128], bf16)
            Wqn = cload(wqn_d, [128, 128], bf16)
            Wv = cload(wv_d, [128, 128], bf16)
            Wp2cx = cload(wp2c_d, [4, 128], bf16)
            Wp2x = cload(wp2_d, [4, 128], bf16)
            BD1 = cload(bd1_d, [128, 128], bf16)
            BD16 = cload(bd16_d, [128, 128], bf16)
            BD2 = cload(bd2_d, [128, 128], bf16)
            I128 = cload(i128_d, [128, 128], bf16)
            BD16f = cload(bd16f_d, [128, 128], f32)
            BD2f = cload(bd2f_d, [128, 128], f32)
            vecs = cload(vecs_d, [128, 6], f32)
            gw1r = vecs[:, 0:1]
            betaw1r = vecs[:, 1:2]
            bw1r = vecs[:, 2:3]
            gw2r = vecs[:, 3:4]
            betaw2r = vecs[:, 4:5]
            epsr = vecs[:, 5:6]

            gidx = big.tile([128, PTS_PAD], i16)
            nc.sync.dma_start(gidx[:], gidx_d[:])
            table = big.tile([128, RANKS, 256], bf16)
            xq = big.tile([128, PTS_PAD], bf16)
            out_sb = big.tile([128, PTS_PAD], f32)

            # ---- table + xq build --------------------------------------
            with (
                tc.tile_pool(name="xtp", bufs=1) as xtp,
                tc.tile_pool(name="psb", bufs=2, space="PSUM") as psb,
            ):
                xT = xtp.tile([128, TPTS], bf16)
                nc.sync.dma_start(xT[:], xT_d[:])
                for grp in range(RANKS // 4):
                    psk = psb.tile([128, 4, 128], f32, tag="psk")
                    psv = psb.tile([128, 4, 128], f32, tag="psv")
                    for r4 in range(4):
                        rk = grp * 4 + r4
                        sl = xT[:, rk * 128:(rk + 1) * 128]
                        nc.tensor.matmul(psk[:, r4, :], sl, Wkc[:])
                        nc.tensor.matmul(psv[:, r4, :], sl, Wv[:])
                    nc.scalar.activation(
                        table[:, grp * 4:(grp + 1) * 4, 0:128], psk[:], AF.Copy)
                    nc.vector.tensor_copy(
                        table[:, grp * 4:(grp + 1) * 4, 128:256], psv[:])
                c0 = 0
                while c0 < PTS_PAD:
                    w_ = min(512, PTS_PAD - c0)
                    pq = psb.tile([128, 512], f32, tag="pq")
                    nc.tensor.matmul(pq[:, :w_], Wqn[:], xT[:, c0:c0 + w_])
                    nc.vector.tensor_copy(xq[:, c0:c0 + w_], pq[:, :w_])
                    c0 += w_

            # ---- main loop ---------------------------------------------
            with (
                tc.tile_pool(name="kvp", bufs=2) as kvp,
                tc.tile_pool(name="work", bufs=2) as work,
                tc.tile_pool(name="ps1", bufs=1, space="PSUM") as ps1,
                tc.tile_pool(name="ps2", bufs=2, space="PSUM") as ps2,
            ):
                if stage == 0:
                    cp = work.tile([128, 128], f32, tag="cp")
                    nc.vector.tensor_copy(cp[:], table[:, 0, 0:128])
                    nc.vector.tensor_copy(out_sb[:, 0:128], cp[:])
                for g in range(NBLK if stage > 0 else 0):
                    BN = 2048
                    kv = kvp.tile([128, 2, BN], bf16, tag="kv")
                    nc.gpsimd.dma_gather(
                        kv[:], table[:], gidx[:, g * 128:(g + 1) * 128],
                        BN, BN, 256, transpose=True,
                        single_packet=False,
                        sbuf_tokens_per_rank=128,
                        sbuf_free_dim_per_rank=512,
                    )
                    if stage >= 2:
                        rz = kvp.tile([4, BN], bf16, tag="rz")
                        nc.sync.dma_start(rz[:], rz4_d[:, g * BN:(g + 1) * BN])
                    if stage == 1:
                        cp = work.tile([128, 512], f32, tag="cp")
                        nc.vector.tensor_copy(cp[:], kv[:, 0, 0:512])
                        nc.vector.tensor_copy(out_sb[:, g*128:g*128+32],
                                              cp[:, 0:32])
                        continue
                    for q in range(4):
                        cs = slice(q * 512, (q + 1) * 512)
                        p0 = g * 128 + q * 32
                        # r (centered) assembled in PSUM
                        ps_rc = ps2.tile([128, 512], f32, tag="rc")
                        nc.tensor.matmul(ps_rc[:], Wp2cx[:], rz[:, cs],
                                         start=True, stop=False)
                        nc.tensor.matmul(ps_rc[:], I128[:], kv[:, 0, cs],
                                         start=False, stop=False)
                        xqb = (xq[:, p0:p0 + 32].unsqueeze(2)
                               .broadcast_to([128, 32, 16]))
                        nc.tensor.matmul(ps_rc[:], I128[:], xqb,
                                         start=False, stop=True)
                        if stage == 2:
                            nc.vector.tensor_copy(out_sb[:, p0:p0+32],
                                                  ps_rc[:, 0:32])
                            continue
                        # LN1
                        sq = work.tile([128, 512], bf16, tag="sq")
                        nc.scalar.activation(sq[:], ps_rc[:], AF.Square)
                        ps_v = ps1.tile([128, 512], f32, tag="v")
                        nc.tensor.matmul(ps_v[:], BD16[:], sq[:])
                        sd = work.tile([128, 512], f32, tag="sd")
                        nc.scalar.activation(sd[:], ps_v[:], AF.Sqrt, bias=epsr)
                        r_ = work.tile([128, 512], f32, tag="r")
                        nc.vector.reciprocal_approx_fast(r_[:], sd[:])
                        t1 = work.tile([128, 512], bf16, tag="t")
                        nc.vector.tensor_tensor(t1[:], ps_rc[:], r_[:],
                                                op=ALU.mult)
                        a1 = work.tile([128, 512], bf16, tag="a")
                        nc.scalar.activation(a1[:], t1[:], AF.Relu,
                                             bias=betaw1r, scale=gw1r)
                        if stage == 3:
                            nc.vector.tensor_copy(out_sb[:, p0:p0+32],
                                                  a1[:, 0:32])
                            continue
                        # W1
                        ps_g = ps2.tile([128, 512], f32, tag="g1")
                        nc.tensor.matmul(ps_g[:], BD1[:], a1[:])
                        # LN2
                        sq2 = work.tile([128, 512], f32, tag="sq2f")
                        nc.scalar.activation(sq2[:], ps_g[:], AF.Square,
                                             bias=bw1r)
                        ps_v2 = ps1.tile([128, 512], f32, tag="v")
                        nc.tensor.matmul(ps_v2[:], BD16f[:], sq2[:])
                        sd2 = work.tile([128, 512], f32, tag="sd")
                        nc.scalar.activation(sd2[:], ps_v2[:], AF.Sqrt,
                                             bias=epsr)
                        r2 = work.tile([128, 512], f32, tag="r")
                        nc.vector.reciprocal_approx_fast(r2[:], sd2[:])
                        t2 = work.tile([128, 512], f32, tag="t2f")
                        nc.vector.scalar_tensor_tensor(
                            t2[:], ps_g[:], bw1r, r2[:],
                            op0=ALU.add, op1=ALU.mult)
                        a2 = work.tile([128, 512], f32, tag="a2f")
                        nc.scalar.activation(a2[:], t2[:], AF.Relu,
                                             bias=betaw2r, scale=gw2r)
                        if stage == 4:
                            nc.vector.tensor_copy(out_sb[:, p0:p0+32],
                                                  a2[:, 0:32])
                            continue
                        # W2 + mean_os (replicated across d)
                        ps_wm = ps1.tile([128, 512], f32, tag="wm")
                        nc.tensor.matmul(ps_wm[:], BD2f[:], a2[:])
                        eb = work.tile([128, 512], f32, tag="eb")
                        nc.scalar.activation(eb[:], ps_wm[:], AF.Exp)
                        # v + pe
                        ps_vp = ps2.tile([128, 512], f32, tag="vp")
                        nc.tensor.matmul(ps_vp[:], Wp2x[:], rz[:, cs],
                                         start=True, stop=False)
                        nc.tensor.matmul(ps_vp[:], I128[:], kv[:, 1, cs],
                                         start=False, stop=True)
                        wv = work.tile([128, 512], f32, tag="wv")
                        nc.vector.tensor_tensor(wv[:], ps_vp[:], eb[:],
                                                op=ALU.mult)
                        # neighbor reduce + normalize
                        eb3 = eb[:].rearrange("p (n s) -> p n s", s=16)
                        wv3 = wv[:].rearrange("p (n s) -> p n s", s=16)
                        es = work.tile([128, 32], f32, tag="es")
                        nc.vector.tensor_reduce(es[:], eb3, axis=AX.X,
                                                op=ALU.add)
                        os_ = work.tile([128, 32], f32, tag="os")
                        nc.vector.tensor_reduce(os_[:], wv3, axis=AX.X,
                                                op=ALU.add)
                        rs = work.tile([128, 32], f32, tag="rs")
                        nc.vector.reciprocal_approx_fast(rs[:], es[:])
                        nc.vector.tensor_tensor(out_sb[:, p0:p0 + 32],
                                                os_[:], rs[:], op=ALU.mult)
                nc.sync.dma_start(out_d[:], out_sb[:])
    nc.compile()
    return nc


# --------------------------------------------------------------------------
# Host-side input preparation
# --------------------------------------------------------------------------

def _headmean(w):
    """Per-head mean over the trailing feature dim (last axis, groups of D)."""
    hm = w.reshape(*w.shape[:-1], H, D).mean(-1, keepdims=True)
    return np.broadcast_to(hm, (*w.shape[:-1], H, D)).reshape(w.shape)


def prepare_in_maps(inputs, cfg=None):
    d = _derive(cfg or DEF_CFG)
    TPTS, PTS_PAD, NBLK, NS_PAD, NPc = (
        d["tpts"], d["pts_pad"], d["nblk"], d["ns_pad"], d["np_"])
    cloud, cpc, n_cores = d["cloud"], d["cores_per_cloud"], d["n_cores"]

    f = lambda a: np.asarray(a, dtype=np.float32)
    p, x, idx = f(inputs["p"]), f(inputs["x"]), np.asarray(inputs["idx"])
    Wq, bq = f(inputs["Wq"]), f(inputs["bq"])
    Wk, bk = f(inputs["Wk"]), f(inputs["bk"])
    Wv, bv = f(inputs["Wv"]), f(inputs["bv"])
    Wp1, bp1 = f(inputs["Wp1"]), f(inputs["bp1"])
    gp, betap = f(inputs["gp"]), f(inputs["betap"])
    Wp2, bp2 = f(inputs["Wp2"]), f(inputs["bp2"])
    gw1, betaw1 = f(inputs["gw1"]), f(inputs["betaw1"])
    Ww1, bw1 = f(inputs["Ww1"]), f(inputs["bw1"])
    gw2, betaw2 = f(inputs["gw2"]), f(inputs["betaw2"])
    Ww2, bw2 = f(inputs["Ww2"]), f(inputs["bw2"])

    # position-encoding 3-vectors (tiny: N*S*3)
    pr = p[idx] - p[:, None, :]                       # (N,S,3)
    y = pr @ Wp1 + bp1
    yc = y - y.mean(-1, keepdims=True)
    v = (yc * yc).mean(-1, keepdims=True)
    z = yc / np.sqrt(v + EPS) * gp + betap
    rz = np.maximum(z, 0.0)                           # (N,S,3)
    rz4 = np.concatenate(
        [rz, np.ones((*rz.shape[:2], 1), np.float32)], -1)  # (N,S,4)

    # folded weights
    Wkc = Wk - _headmean(Wk)
    Wqn = -(Wq - _headmean(Wq))
    Wp2c = Wp2 - _headmean(Wp2)
    const_c = ((bp2 - _headmean(bp2)) + (bk - _headmean(bk))
               - (bq - _headmean(bq)))                # (128,)
    Wp2cx = np.concatenate([Wp2c, const_c[None, :]], 0)   # (4,128)
    Wp2x = np.concatenate([Wp2, (bp2 + bv)[None, :]], 0)  # (4,128)

    Ww1c = Ww1 - Ww1.mean(-1, keepdims=True)   # pre-center LN2's input
    bw1c = bw1 - bw1.mean()
    BD1 = np.zeros((128, 128), np.float32)
    BD16 = np.zeros((128, 128), np.float32)
    BD2 = np.zeros((128, 128), np.float32)
    W2bar = Ww2.mean(-1)                              # (16,)
    for h in range(H):
        sl = slice(h * D, (h + 1) * D)
        BD1[sl, sl] = Ww1c
        BD16[sl, sl] = 1.0 / D
        BD2[sl, sl] = np.broadcast_to(W2bar[:, None], (D, D))
    I128 = np.eye(128, dtype=np.float32)

    vecs = np.zeros((128, 6), np.float32)
    vecs[:, 0] = np.tile(gw1, H)
    vecs[:, 1] = np.tile(betaw1, H)
    vecs[:, 2] = np.tile(bw1c, H)
    vecs[:, 3] = np.tile(gw2, H)
    vecs[:, 4] = np.tile(betaw2, H)
    vecs[:, 5] = EPS

    wts = dict(
        Wkc=Wkc.astype(BF16), Wqn=Wqn.astype(BF16), Wv=Wv.astype(BF16),
        Wp2cx=Wp2cx.astype(BF16), Wp2x=Wp2x.astype(BF16),
        BD1=BD1.astype(BF16), BD16=BD16.astype(BF16), BD2=BD2.astype(BF16),
        I128=I128.astype(BF16), BD16f=BD16, BD2f=BD2, vecs=vecs,
    )

    in_maps = []
    for c in range(n_cores):
        g0, g1 = c * NPc, (c + 1) * NPc
        base = (c // cpc) * cloud
        perm = np.concatenate([
            np.arange(g0, g1), np.arange(base, g0),
            np.arange(g1, base + cloud)])
        xTc = np.zeros((TPTS, 128), BF16)
        xTc[:cloud] = x[perm].astype(BF16)
        inv = np.empty(cloud, np.int64)
        inv[perm - base] = np.arange(cloud)
        li = inv[idx[g0:g1].reshape(-1) - base]
        li_pad = np.zeros(NS_PAD, np.int64)
        li_pad[:NPc * S] = li
        A16 = (li_pad.reshape(NBLK, 128, S).transpose(0, 2, 1)
               .swapaxes(0, 1).reshape(S, NBLK * 128))
        gidx = np.tile(A16, (128 // S, 1)).astype(np.int16)
        rzc = np.zeros((NS_PAD, 4), np.float32)
        rzc[:NPc * S] = rz4[g0:g1].reshape(-1, 4)
        in_maps.append(dict(
            xT=np.ascontiguousarray(xTc.T),
            rz4=np.ascontiguousarray(rzc.T.astype(BF16)),
            gidx=np.ascontiguousarray(gidx),
            **wts,
        ))
    return in_maps


# --------------------------------------------------------------------------
# Cached executor (axon/PJRT path, mirrors bass2jax.run_bass_via_pjrt)
# --------------------------------------------------------------------------

_STATE = {}


def _get_exec(cfg=None):
    import os as _os
    key = (tuple(sorted((cfg or DEF_CFG).items())),
           _os.environ.get("BASS_STAGE", "5"))
    if key in _STATE:
        return _STATE[key]

    import jax
    import concourse.mybir as mybir
    from concourse import bass2jax
    from concourse.bass_interp import get_hw_module
    from jax.sharding import Mesh, PartitionSpec
    from jax.experimental.shard_map import shard_map

    d = _derive(cfg or DEF_CFG)
    n_cores = d["n_cores"]

    nc = build_module(cfg)
    nc.m = get_hw_module(nc.m)
    bass2jax.install_neuronx_cc_hook()

    part_name = (nc.partition_id_tensor.name
                 if nc.partition_id_tensor is not None else None)
    in_names, out_names, out_avals = [], [], []
    for alloc in nc.m.functions[0].allocations:
        if not isinstance(alloc, mybir.MemoryLocationSet):
            continue
        name = alloc.memorylocations[0].name
        if alloc.kind == "ExternalInput":
            if name != part_name:
                in_names.append(name)
        elif alloc.kind == "ExternalOutput":
            out_names.append(name)
            out_avals.append(jax.core.ShapedArray(
                tuple(alloc.tensor_shape), mybir.dt.np(alloc.dtype)))
    n_params = len(in_names)
    n_outs = len(out_avals)
    all_names = in_names + out_names + ([part_name] if part_name else [])
    donate = tuple(range(n_params, n_params + n_outs))

    def _body(*args):
        operands = list(args)
        if part_name is not None:
            operands.append(bass2jax.partition_id_tensor())
        outs = bass2jax._bass_exec_p.bind(
            *operands,
            out_avals=tuple(out_avals),
            in_names=tuple(all_names),
            out_names=tuple(out_names),
            lowering_input_output_aliases=(),
            sim_require_finite=True,
            sim_require_nnan=True,
            nc=nc,
        )
        return tuple(outs)

    devices = jax.devices()[:n_cores]
    if n_cores == 1:
        fn = jax.jit(_body, donate_argnums=donate, keep_unused=True)
    else:
        mesh = Mesh(np.asarray(devices), ("core",))
        fn = jax.jit(
            shard_map(_body, mesh=mesh,
                      in_specs=(PartitionSpec("core"),) * (n_params + n_outs),
                      out_specs=(PartitionSpec("core"),) * n_outs,
                      check_rep=False),
            donate_argnums=donate, keep_unused=True)

    st = dict(fn=fn, in_names=in_names, out_names=out_names,
              out_avals=out_avals, n_cores=n_cores, d=d)
    _STATE[key] = st
    return st


def run_device(in_maps, cfg=None):
    """Run the SPMD kernel; returns per-core dict of outputs."""
    st = _get_exec(cfg)
    n_cores = st["n_cores"]
    concat_in = [
        np.concatenate([np.asarray(m[name]) for m in in_maps], axis=0)
        for name in st["in_names"]
    ]
    zeros = [np.zeros((n_cores * a.shape[0], *a.shape[1:]), a.dtype)
             for a in st["out_avals"]]
    out_arrs = st["fn"](*concat_in, *zeros)
    res = []
    for c in range(n_cores):
        res.append({
            name: np.asarray(out_arrs[i]).reshape(
                n_cores, *st["out_avals"][i].shape)[c]
            for i, name in enumerate(st["out_names"])
        })
    return res


def kernel(**inputs):
    d = _derive(DEF_CFG)
    NPc = d["np_"]
    in_maps = prepare_in_maps(inputs)
    res = run_device(in_maps)
    out = np.empty((N, O), np.float32)
    for c in range(d["n_cores"]):
        out[c * NPc:(c + 1) * NPc] = res[c]["outT"][:, :NPc].T
    return out
